# revision 17
# baseline (speedup 1.0000x reference)
"""DiceBCELossWithTopology fused loss kernel for Trainium2 (8 NeuronCores).

Reference computation (on inputs x, t of shape (64,1,512,512) f32, flattened):
  dice  = 1 - (2*sum(x*t)+1) / (sum(x)+sum(t)+1)
  bce   = mean(-(t*max(log x,-100) + (1-t)*max(log1p(-x),-100)))
  topo  = |n_runs_of_nonzero(x) - 1| / (512*512)
  loss  = 0.5*bce + dice + topo

Strategy (data-parallel over 8 cores, memory-bound -> ship bf16):
  The device math was already bf16 end-to-end for every t-term (t cast to
  bf16 weights) and for sum(x)/sum(x*t), so the host pre-casts x and t to
  bf16 and halves HBM traffic (16.8 -> 8.4 MB/core).  The one real
  casualty is log1p(-x) for x in (1-2^-9, 1): bf16 rounds those to 1.0
  and Ln(1-x) blows up.  The host clamps them to the largest bf16 < 1
  (0.99609375) and adds exact f64 corrections for their bce / dice
  contributions (~32k of 16.7M elements).

  With DMA halved the scalar engine (2 Ln passes) is the wall, so all
  reductions ride the PE: sub-chunks are 127 data columns wide and every
  matmul's 128th weight column is all-ones, making PSUM row 127
  accumulate the plain column sums (sum L2, sum x) for free - no
  activation accumulator reads, no separate ones-weight matmuls.  Rows
  are zero-padded from 16384 to 130*127=16510 columns (zero pads are
  exactly neutral for every statistic: t=0 kills the t-terms, Ln(1-0)=0
  kills sum(L2), x=0 kills sum(x)/sum(x*t) and creates no run starts).

  Each core streams its padded [128, 16510] bf16 shard in chunks
  (4-deep rotation).  Per chunk:
    ACT : L1 = Ln(x), L2 = Ln(1-x) into strided R groups [L1|ones|L2].
    DVE : starts = (x_prev==0)&(x_cur!=0) via one STT (accum_out -> free
          run-start count), clamp L1 to -100 in place (4x mode).
    PE  : per 127-col sub-chunk with lhsT = [t-cols | ones]: mm over R
          (255 free: diag t.L1c, ones col sum(t), diag t.L2, row 127
          sum(L2)) + mm over x-cols (diag x.t, row 127 sum(x)), PSUM
          ping-pong accumulated.
  Host: float64 final reduction over tiny per-core stats + row/shard
  boundary run-start corrections + near-1 clamp corrections + assembly.

bf16(x) == 0 iff x == 0 for this input domain (no f32 denormals), so
topology is exact.  log(x) hits -inf only at x == 0 (and the zero pads);
the DVE max(L1, -100) clamp maps -inf -> -100 exactly.
"""

import numpy as np

# Problem constants (hardcoded per harness contract - no file reads here).
N_CORES = 8
P = 128                      # SBUF partitions
COLS = 16384                 # real columns per core: 2M elements / 128
W = 127                      # data columns per sub-chunk (col 127 = ones)
NG = 130                     # sub-chunk groups per row: 130*127 = 16510
XCOLS = NG * W               # padded x columns per row
TCOLS = NG * (W + 1)         # interleaved t columns per row
# Chunk sizes in groups: small head so compute starts early, tapered tail.
CHUNKS_G = [2, 6, 12, 24, 32, 32, 16, 4, 2]
NHOIST = 3                   # chunks whose x-DMA issues before any t-DMA
NCHUNK = len(CHUNKS_G)
TOTAL = 64 * 512 * 512       # 16_777_216 elements
IMAGE_PIXELS = 512 * 512
SMOOTH = 1.0
LOG_CLAMP = -100.0
BCE_WEIGHT = 0.5
TOPOLOGY_WEIGHT = 1.0

# R group layout: [L1 0:127 | ones 127 | L2 128:255 | pad]
GW = 256                     # group stride
NRHS = 255                   # psumB matmul free size

XCLAMP = 0.99609375          # largest bf16 < 1.0 (= 1 - 2^-8)

_CACHE = {}


def _build_nc():
    from concourse.bacc import Bacc
    import concourse.mybir as mybir
    from concourse.tile import TileContext

    F32 = mybir.dt.float32
    BF16 = mybir.dt.bfloat16
    AF = mybir.ActivationFunctionType
    OP = mybir.AluOpType
    AX = mybir.AxisListType

    assert sum(CHUNKS_G) == NG

    nc = Bacc()
    x_d = nc.dram_tensor("x", [P, XCOLS], BF16, kind="ExternalInput")
    t_d = nc.dram_tensor("t", [P, TCOLS], BF16, kind="ExternalInput")
    eye_d = nc.dram_tensor("eye", [P, W], F32, kind="ExternalInput")
    stats_d = nc.dram_tensor("stats", [P, 32], F32, kind="ExternalOutput")

    with TileContext(nc) as tc:
        with tc.tile_pool(name="const", bufs=1) as cpool, \
             tc.tile_pool(name="work", bufs=4) as pool, \
             tc.tile_pool(name="psum", bufs=1, space="PSUM") as psum_pool:

            eye = cpool.tile([P, W], F32)
            stats = cpool.tile([P, 32], F32)

            # Ping-pong PSUM banks: matmul N into bank (N%2) overlaps its
            # drain with matmul N+1's fill.
            psumB = [psum_pool.tile([P, NRHS], F32, name=f"psumB{i}")
                     for i in range(2)]     # t-dots + sum(t) + row127 sums
            psumXT = [psum_pool.tile([P, W], F32, name=f"psumXT{i}")
                      for i in range(2)]    # diag x.t + row127 sum(x)

            GMAX = max(CHUNKS_G)
            ci = 0        # global sub-chunk index
            goff = 0      # group offset

            # Hoisted x-DMAs for the first chunks: ACT only needs x, and the
            # ~600ns serialized issue cost of interleaved t-DMAs would
            # otherwise starve the scalar engine during pipeline ramp.
            xts, tts = [], []
            hg = 0
            for j in range(NHOIST):
                G = CHUNKS_G[j]
                FC = G * W
                off = hg * W
                x_t = pool.tile([P, GMAX * W + 1], BF16, tag="x_t",
                                name=f"x_t{j}")[:, :FC + 1]
                if j == 0:
                    nc.sync.dma_start(x_t[:, 1:FC + 1], x_d[:, 0:FC])
                    nc.vector.memset(x_t[:, 0:1], 1.0)  # no phantom run start
                else:
                    nc.sync.dma_start(x_t, x_d[:, off - 1:off + FC])
                xts.append(x_t)
                hg += G
            hg = 0
            for j in range(NHOIST):
                G = CHUNKS_G[j]
                t_t = pool.tile([P, GMAX * (W + 1)], BF16, tag="t_t",
                                name=f"t_t{j}")[:, :G * (W + 1)]
                nc.sync.dma_start(
                    t_t, t_d[:, hg * (W + 1):(hg + G) * (W + 1)])
                tts.append(t_t)
                hg += G
            nc.sync.dma_start(eye[:], eye_d[:])
            nc.vector.memset(stats[:], 0.0)

            for j, G in enumerate(CHUNKS_G):
                FC = G * W
                off = goff * W
                if j < NHOIST:
                    x_t, t_t = xts[j], tts[j]
                else:
                    x_t = pool.tile([P, GMAX * W + 1], BF16, tag="x_t",
                                    name=f"x_t{j}")[:, :FC + 1]
                    t_t = pool.tile([P, GMAX * (W + 1)], BF16, tag="t_t",
                                    name=f"t_t{j}")[:, :G * (W + 1)]
                    nc.sync.dma_start(x_t, x_d[:, off - 1:off + FC])
                    nc.sync.dma_start(
                        t_t, t_d[:, goff * (W + 1):(goff + G) * (W + 1)])
                R = pool.tile([P, GMAX * GW], BF16,
                              tag="R", name=f"R{j}")[:, :G * GW]
                st = pool.tile([P, GMAX * W], BF16, tag="st",
                               name=f"st{j}")[:, :FC]

                x3 = x_t[:, 1:FC + 1].rearrange("p (g w) -> p g w", w=W)
                R3 = R.rearrange("p (g w) -> p g w", w=GW)

                # ---- ACT: logs (bf16 out), no accumulator reads
                nc.scalar.activation(R3[:, :, 0:W], x3, AF.Ln)
                nc.scalar.activation(R3[:, :, W + 1:2 * W + 1], x3, AF.Ln,
                                     scale=-1.0, bias=1.0)

                # ---- DVE: ones col, fused run-start detect+count, clamp
                nc.vector.memset(R3[:, :, W:W + 1], 1.0)
                # starts = (x_prev == 0) & (x_cur != 0), summed for free
                nc.vector.scalar_tensor_tensor(
                    out=st, in0=x_t[:, 0:FC], scalar=0.0,
                    in1=x_t[:, 1:FC + 1], op0=OP.is_equal, op1=OP.logical_and,
                    accum_out=stats[:, 20 + j:21 + j])
                nc.vector.tensor_scalar(R3[:, :, 0:W], R3[:, :, 0:W],
                                        LOG_CLAMP, None, OP.max)

                # ---- PE: per sub-chunk: fused-dots matmul + x.t matmul.
                # The last chunk goes entirely to bank 1 so bank 0's
                # extraction overlaps the final chunks' compute.
                LAST0 = NG - CHUNKS_G[-1] - 1 - (NG - CHUNKS_G[-1] - 1) % 2
                for c in range(G):
                    bank = 1 if ci >= NG - CHUNKS_G[-1] else ci % 2
                    first = ci < 2
                    last = ci == LAST0 or ci == NG - 1
                    lhsT = t_t[:, c * (W + 1):(c + 1) * (W + 1)]
                    nc.tensor.matmul(
                        psumB[bank][:], lhsT,
                        R[:, c * GW:c * GW + NRHS],
                        start=first, stop=last, skip_group_check=True)
                    nc.tensor.matmul(
                        psumXT[bank][:], lhsT,
                        x_t[:, c * W + 1:(c + 1) * W + 1],
                        start=first, stop=last, skip_group_check=True)
                    ci += 1
                goff += G
                if j == NCHUNK - 2:
                    # bank 0 is final here - drain it while the tail runs
                    psB_sb = cpool.tile([P, NRHS], F32, name="psB_sb")
                    psXT_sb = cpool.tile([P, W], F32, name="psXT_sb")
                    nc.vector.tensor_copy(psB_sb[:], psumB[0][:])
                    nc.vector.tensor_copy(psXT_sb[:], psumXT[0][:])

            # ---- extraction tail: fold bank 1 in, then fused diag reduces
            # on DVE while ACT does the plain row sums in parallel.
            scr = cpool.tile([P, W], F32)
            scr2 = cpool.tile([P, W], F32)
            scr3 = cpool.tile([P, W], F32)
            nc.vector.tensor_tensor(psB_sb[:], psB_sb[:], psumB[1][:], OP.add)
            nc.vector.tensor_tensor(psXT_sb[:], psXT_sb[:], psumXT[1][:], OP.add)
            nc.vector.tensor_tensor(scr[:], psB_sb[:, 0:W], eye[:], OP.mult)
            nc.vector.tensor_reduce(stats[:, 0:1], scr[:], AX.X, OP.add)   # t.L1c
            nc.vector.tensor_copy(stats[:, 1:2], psB_sb[:, W:W + 1])       # sum t
            nc.vector.tensor_tensor(scr2[:], psB_sb[:, W + 1:2 * W + 1],
                                    eye[:], OP.mult)
            nc.vector.tensor_reduce(stats[:, 2:3], scr2[:], AX.X, OP.add)  # t.L2
            nc.vector.tensor_tensor(scr3[:], psXT_sb[:], eye[:], OP.mult)
            nc.vector.tensor_reduce(stats[:, 3:4], scr3[:], AX.X, OP.add)  # x.t
            # plain column sums per partition; host reads row 127
            # (the ones-weight row): sum(L2) and sum(x)
            nc.vector.tensor_reduce(stats[:, 8:9],
                                    psB_sb[:, W + 1:2 * W + 1], AX.X, OP.add)
            nc.vector.tensor_reduce(stats[:, 5:6], psXT_sb[:], AX.X, OP.add)
            nc.sync.dma_start(stats_d[:], stats[:])

    nc.finalize()
    return nc


def _get_nc():
    if "nc" not in _CACHE:
        _CACHE["nc"] = _build_nc()
    return _CACHE["nc"]


def _prep(inputs: np.ndarray, targets: np.ndarray):
    """Host-side: flatten, bf16-cast with near-1 clamp, pad/interleave,
    shard, and compute the exact f64 corrections for clamped elements."""
    import ml_dtypes
    BF = ml_dtypes.bfloat16

    xf = np.ascontiguousarray(inputs, dtype=np.float32).reshape(-1)
    tf = np.ascontiguousarray(targets, dtype=np.float32).reshape(-1)
    assert xf.size == TOTAL and tf.size == TOTAL

    xb = xf.astype(BF)
    tb = tf.astype(BF)
    mask = xb == np.float32(1.0)          # elements whose L2 would be Ln(0)
    idx = np.nonzero(mask)[0]
    if idx.size:
        xb[idx] = np.float32(XCLAMP)

    # Exact f64 corrections for the clamped elements: the device computes
    # with x_hat = XCLAMP and the exact bf16 constants below; the reference
    # uses f32 x.  (t is bf16 on both sides, so use bf16(t) in corrections.)
    corr = {"t1": 0.0, "t2": 0.0, "l2": 0.0, "sx": 0.0, "xt": 0.0}
    if idx.size:
        xm = xf[idx].astype(np.float64)
        tm = tb[idx].astype(np.float64)
        l1_dev = float(np.float32(BF(np.log(np.float32(XCLAMP)))))
        l2_dev = float(np.float32(BF(np.log1p(np.float32(-XCLAMP)))))
        corr["t1"] = float(np.sum(tm * (np.log(xm) - l1_dev)))
        corr["t2"] = float(np.sum(tm * (np.log1p(-xm) - l2_dev)))
        corr["l2"] = float(np.sum(np.log1p(-xm) - l2_dev))
        corr["sx"] = float(np.sum(xm - XCLAMP))
        corr["xt"] = float(np.sum(tm * (xm - XCLAMP)))

    eye = np.eye(P, W, dtype=np.float32)
    shard = TOTAL // N_CORES
    in_maps = []
    for c in range(N_CORES):
        xr = xb[c * shard:(c + 1) * shard].reshape(P, COLS)
        tr = tb[c * shard:(c + 1) * shard].reshape(P, COLS)
        xp = np.zeros((P, XCOLS), dtype=BF)
        xp[:, :COLS] = xr
        t3 = np.zeros((P, NG, W + 1), dtype=BF)
        tp = np.zeros((P, XCOLS), dtype=BF)
        tp[:, :COLS] = tr
        t3[:, :, :W] = tp.reshape(P, NG, W)
        t3[:, :, W] = np.float32(1.0)
        in_maps.append({
            "x": xp,
            "t": t3.reshape(P, TCOLS),
            "eye": eye,
        })
    return xf, in_maps, corr


def kernel(inputs: np.ndarray, targets: np.ndarray) -> np.ndarray:
    from concourse.bass_utils import run_bass_kernel_spmd

    xf, in_maps, corr = _prep(inputs, targets)

    nc = _get_nc()
    res = None
    for attempt in range(3):
        try:
            res = run_bass_kernel_spmd(nc, in_maps, core_ids=list(range(N_CORES)))
            break
        except Exception:
            if attempt == 2:
                raise
    assert res is not None

    s_xt = s_x = s_t = t1 = t2 = s_l2 = 0.0
    n_starts = 0.0
    for c in range(N_CORES):
        stt = res.results[c]["stats"].astype(np.float64)
        t1 += stt[:127, 0].sum()
        s_t += stt[:127, 1].sum()
        t2 += stt[:127, 2].sum()
        s_xt += stt[:127, 3].sum()
        s_x += stt[127, 5]
        s_l2 += stt[127, 8]
        n_starts += stt[:, 20:20 + NCHUNK].sum()

    t1 += corr["t1"]
    t2 += corr["t2"]
    s_l2 += corr["l2"]
    s_x += corr["sx"]
    s_xt += corr["xt"]

    # Host-side boundary run starts: row boundaries (incl. shard cuts) and
    # the first element.  1023 pairs + 1 element - O(1) work.
    prev = xf[COLS - 1:-1:COLS]
    cur = xf[COLS::COLS]
    n_starts += np.count_nonzero((cur != 0) & (prev == 0))
    n_starts += float(xf[0] != 0)

    dice = 1.0 - (2.0 * s_xt + SMOOTH) / (s_x + s_t + SMOOTH)
    bce = -(t1 - t2 + s_l2) / TOTAL
    topo = abs(n_starts - 1.0) / IMAGE_PIXELS
    loss = bce * BCE_WEIGHT + dice + topo * TOPOLOGY_WEIGHT
    return np.array(loss, dtype=np.float32)
